# revision 2
# baseline (speedup 1.0000x reference)
"""Trainium2 Bass kernel for nn_Attention (B=4, S=2048, HIDDEN=768, 12 heads).

Sharding: 8 cores = 4 batches x 2 head-groups (6 heads each). Projection
weights are sliced per head-group and pre-transposed on the host; the
1/sqrt(64) scale is folded into Wq. Each core computes a partial output
(its head-group's contribution through Wo, with bo/2 bias); the host sums
the two partials per batch.

Projection matmuls run in float32r (fp32 with 11-bit mantissa): single-pass
on the PE, accumulating in fp32 PSUM. The attention inner product
attn += v^T @ exp(scores) runs in fp8e4 DoubleRow mode (two k-chunks per
pass, 2 MACs/cell/cycle): exp writes fp8 E pairs side by side in SBUF and
the augmented v^T (ones-column per head appends the softmax denominator)
is stored fp8 with a 16B-aligned chunk stride.

Per-core device program:
  warmup: dense matmul burst at kernel start un-throttles the PE HAM clock
  q,k  : [384, S] via matmuls with pre-transposed weights (o on partitions)
  vT   : [S, 6x65 cols] computed directly transposed, stored fp8e4
  scores S_c = k_chunk^T q -> PSUM [128, QT]   (k on partitions, q on free)
  E_c = exp(S_c) on ScalarE -> fp8 pair slot   (no max subtraction; |s| < ~3)
  A  += DoubleRow([vT_2t|vT_2t+1]^T @ [E_2t|E_2t+1]) -> PSUM [65, QT]
  u65 = copy(A) -> SBUF (single 65-row evacuation frees PSUM fast)
  den rows DMA-gathered to DRAM; one batched reciprocal [6, QT]; DMA
  broadcast back; attn = u65[0:64] * bcast(1/den)
  out_partial = WoT_g^T @ attn + bo/2
"""

import numpy as np

HIDDEN = 768
NUM_HEADS = 12
D = 64
B = 4
S = 2048
P = 128

H_CORE = 6          # heads per core
OC = H_CORE * D     # 384 output channels per core for q/k
WAUG = H_CORE * (D + 1)  # 390: v columns with interleaved ones-columns
VSTRIDE = 400       # padded v_t chunk stride (16B aligned for DoubleRow AP)
C_CHUNKS = HIDDEN // P   # 6
QT = 1024           # q-tile (free dim) for the attention inner loop
MMF = 512           # max fp32 moving free dim per matmul
TPAIRS = 8          # k-chunk pairs per q-tile (16 chunks of 128 -> 8 pairs)

_CACHE = {}


def _round_f32r(a: np.ndarray) -> np.ndarray:
    """Round fp32 array to float32r (11-bit mantissa) with round-to-nearest-even."""
    u = np.ascontiguousarray(a, dtype=np.float32).view(np.uint32)
    lsb = (u >> np.uint32(12)) & np.uint32(1)
    r = ((u + np.uint32(0x7FF) + lsb) >> np.uint32(12)) << np.uint32(12)
    return r.view(np.float32)


def _build(with_mask: bool):
    import concourse.bass as bass
    import concourse.tile as tile
    from concourse import bacc, mybir
    from contextlib import ExitStack

    f32 = mybir.dt.float32
    f32r = mybir.dt.float32r
    bf16 = mybir.dt.bfloat16
    f8 = mybir.dt.float8e4
    AF = mybir.ActivationFunctionType
    ALU = mybir.AluOpType
    DR = mybir.MatmulPerfMode.DoubleRow

    nc = bacc.Bacc(
        "TRN2",
        target_bir_lowering=False,
        debug=False,
        enable_asserts=True,
        num_devices=8,
    )

    x_d = nc.dram_tensor("x", (HIDDEN, S), f32r, kind="ExternalInput").ap()
    wq_d = nc.dram_tensor("wqT", (HIDDEN, OC), f32r, kind="ExternalInput").ap()
    bq_d = nc.dram_tensor("bq", (OC,), f32, kind="ExternalInput").ap()
    wk_d = nc.dram_tensor("wkT", (HIDDEN, OC), f32r, kind="ExternalInput").ap()
    bk_d = nc.dram_tensor("bk", (OC,), f32, kind="ExternalInput").ap()
    wv_d = nc.dram_tensor("wvT", (HIDDEN, WAUG), f32r, kind="ExternalInput").ap()
    bvb_d = nc.dram_tensor("bvb", (P, WAUG), f32, kind="ExternalInput").ap()
    wo_d = nc.dram_tensor("woT", (OC, HIDDEN), f32r, kind="ExternalInput").ap()
    bo_d = nc.dram_tensor("bo", (HIDDEN,), f32, kind="ExternalInput").ap()
    if with_mask:
        em_d = nc.dram_tensor("expmask", (S, S), f32, kind="ExternalInput").ap()
    out_d = nc.dram_tensor("out", (HIDDEN, S), f32, kind="ExternalOutput").ap()

    QTILES = OC // P      # 3 q/k sbuf tiles
    STILES = S // P       # 16 s-position chunks
    NQH = S // QT         # 2 q-halves
    NF = QT // MMF        # 2 matmul free-slices per QT

    x_r = x_d.rearrange("(t p) s -> p t s", p=P)
    wq_r = wq_d.rearrange("(t p) o -> p t o", p=P)
    wk_r = wk_d.rearrange("(t p) o -> p t o", p=P)
    wv_r = wv_d.rearrange("(t p) o -> p t o", p=P)
    wo_r = wo_d.rearrange("(t p) o -> p t o", p=P)
    bq_r = bq_d.rearrange("(t p) -> p t", p=P)
    bk_r = bk_d.rearrange("(t p) -> p t", p=P)
    bo_r = bo_d.rearrange("(t p) -> p t", p=P)
    out_r = out_d.rearrange("(t p) s -> p t s", p=P)

    with tile.TileContext(nc) as tc, ExitStack() as ctx:
        consts = ctx.enter_context(tc.tile_pool(name="consts", bufs=1))
        persist = ctx.enter_context(tc.tile_pool(name="persist", bufs=1))

        wo_t = consts.tile([P, QTILES, HIDDEN], f32r)
        nc.sync.dma_start(wo_t[:], wo_r)
        bq_t = consts.tile([P, QTILES], f32)
        nc.sync.dma_start(bq_t[:], bq_r)
        bk_t = consts.tile([P, QTILES], f32)
        nc.sync.dma_start(bk_t[:], bk_r)
        bo_t = consts.tile([P, C_CHUNKS], f32)
        nc.sync.dma_start(bo_t[:], bo_r)
        bvb_t = consts.tile([P, WAUG], f32)
        nc.sync.dma_start(bvb_t[:], bvb_d)

        q_t = persist.tile([P, QTILES, S], bf16)
        k_t = persist.tile([P, QTILES, S], bf16)
        v_t = persist.tile([P, STILES, VSTRIDE], f8)
        attn_t = persist.tile([P, QTILES, S], f32r)

        # ---------------- phase A: projections ----------------
        with (
            tc.tile_pool(name="phA", bufs=1) as phA,
            tc.tile_pool(name="psA", bufs=2, space="PSUM") as psA,
        ):
            # Startup warmup: the PE starts HAM-throttled at 1.2 GHz and only
            # un-throttles after ~3.4us of sustained activity. Burn that time
            # on dummy matmuls while the x/w DMAs land, so the real
            # projection matmuls run at 2.4 GHz.
            wu0 = psA.tile([P, QT], f32, tag="Pq", name="warmup0")
            for i in range(20):
                nc.tensor.matmul(
                    wu0[:, 0:MMF], wo_t[:, 0, 0:P], wo_t[:, 0, 0:MMF],
                    start=True, stop=True,
                )

            x_t = phA.tile([P, C_CHUNKS, S], f32r)
            wq_t = phA.tile([P, C_CHUNKS, OC], f32r)
            wk_t = phA.tile([P, C_CHUNKS, OC], f32r)
            wv_t = phA.tile([P, C_CHUNKS, WAUG], f32r)
            for c in range(C_CHUNKS):
                nc.sync.dma_start(wq_t[:, c, :], wq_r[:, c, :])
                nc.sync.dma_start(wk_t[:, c, :], wk_r[:, c, :])
                nc.sync.dma_start(x_t[:, c, :], x_r[:, c, :])
                nc.sync.dma_start(wv_t[:, c, :], wv_r[:, c, :])

            # q, k projections: out[o_tile(128), s] = sum_c WT[c,o]^T x[c,s]
            # tile-interleaved so q/k tile 0 complete early
            for ot in range(QTILES):
                for dst, w_sb, b_sb in ((q_t, wq_t, bq_t), (k_t, wk_t, bk_t)):
                    for half in range(S // QT):
                        ps = psA.tile([P, QT], f32, tag="Pq")
                        for c in range(C_CHUNKS):
                            for nf in range(NF):
                                nc.tensor.matmul(
                                    ps[:, nf * MMF:(nf + 1) * MMF],
                                    w_sb[:, c, ot * P:(ot + 1) * P],
                                    x_t[:, c, half * QT + nf * MMF:
                                        half * QT + (nf + 1) * MMF],
                                    start=(c == 0),
                                    stop=(c == C_CHUNKS - 1),
                                )
                        nc.vector.tensor_scalar_add(
                            dst[:, ot, half * QT:(half + 1) * QT],
                            ps[:],
                            b_sb[:, ot:ot + 1],
                        )

            # vT projection: out[s_tile(128), 390] = sum_c x[c,s]^T WvT[c,:]
            for st in range(STILES):
                ps = psA.tile([P, WAUG], f32, tag="Pv")
                for c in range(C_CHUNKS):
                    nc.tensor.matmul(
                        ps[:],
                        x_t[:, c, st * P:(st + 1) * P],
                        wv_t[:, c, :],
                        start=(c == 0),
                        stop=(c == C_CHUNKS - 1),
                    )
                nc.vector.tensor_tensor(
                    v_t[:, st, 0:WAUG], ps[:], bvb_t[:], ALU.add
                )

        # ---------------- phase B: attention ----------------
        with (
            tc.tile_pool(name="phB", bufs=4) as phB,
            tc.tile_pool(name="psB", bufs=2, space="PSUM") as psB,
            tc.tile_pool(name="outp", bufs=2) as outp,
            tc.tile_pool(name="dscr", bufs=2, space="DRAM") as dscr,
        ):
            # re-warm after the proj->attention boundary
            wu = psB.tile([P, QT], f32, tag="S", name="warmup")
            for i in range(14):
                nc.tensor.matmul(
                    wu[:, 0:MMF], q_t[:, 0, 0:P], q_t[:, 0, 0:MMF],
                    start=True, stop=True,
                )
            for qh in range(NQH):
                us = []
                scr = dscr.tile([H_CORE, QT], f32, name=f"scr{qh}")
                for hp in range(H_CORE // 2):
                    heads = (2 * hp, 2 * hp + 1)
                    accs = [
                        psB.tile([D + 1, QT], f32, tag="A", name=f"acc{i}")
                        for i in range(2)
                    ]
                    for t in range(TPAIRS):
                        edrs = []
                        for hi, h in enumerate(heads):
                            pb = 64 * (h % 2)
                            e_dr = phB.tile([P, 2, QT], f8, tag="E")
                            for cp in range(2):
                                c = 2 * t + cp
                                sc = psB.tile([P, QT], f32, tag="S")
                                for nf in range(NF):
                                    nc.tensor.matmul(
                                        sc[:, nf * MMF:(nf + 1) * MMF],
                                        k_t[pb:pb + D, h // 2,
                                            c * P:(c + 1) * P],
                                        q_t[pb:pb + D, h // 2,
                                            qh * QT + nf * MMF:
                                            qh * QT + (nf + 1) * MMF],
                                        start=True,
                                        stop=True,
                                    )
                                nc.scalar.activation(
                                    e_dr[:, cp, :], sc[:], AF.Exp
                                )
                                if with_mask:
                                    em = phB.tile([P, QT], f32, tag="M")
                                    nc.sync.dma_start(
                                        em[:],
                                        em_d[c * P:(c + 1) * P,
                                             qh * QT:(qh + 1) * QT],
                                    )
                                    nc.vector.tensor_tensor(
                                        e_dr[:, cp, :], e_dr[:, cp, :],
                                        em[:], ALU.mult
                                    )
                            edrs.append(e_dr)
                        for hi, h in enumerate(heads):
                            for nf in range(NF):
                                nc.tensor.matmul(
                                    accs[hi][:, nf * MMF:(nf + 1) * MMF],
                                    v_t[:, 2 * t:2 * t + 2,
                                        65 * h:65 * h + 65],
                                    edrs[hi][:, :, nf * MMF:(nf + 1) * MMF],
                                    start=(t == 0),
                                    stop=(t == TPAIRS - 1),
                                    perf_mode=DR,
                                )
                    for hi, h in enumerate(heads):
                        # single 65-row evacuation frees the acc PSUM fast;
                        # row 64 (softmax denominator) is DMA-gathered to DRAM
                        u65 = phB.tile([D + 1, QT], f32, tag="U", bufs=7,
                                       name=f"u{h}")
                        nc.vector.tensor_copy(u65[:], accs[hi][:])
                        nc.sync.dma_start(scr[h:h + 1, :], u65[D:D + 1, :])
                        us.append((h, u65))
                    # hp-boundary warmup: the acc-tag FIFO stalls the PE here
                    # long enough for HAM to re-throttle; keep it busy.
                    wub = psB.tile([P, QT], f32, tag="S", name=f"wub{qh}_{hp}")
                    for i in range(5):
                        nc.tensor.matmul(
                            wub[:, 0:MMF], q_t[:, 0, 0:P], q_t[:, 0, 0:MMF],
                            start=True, stop=True,
                        )
                # batched reciprocal of all 6 denominators of this qh
                dens = phB.tile([H_CORE, QT], f32, tag="dn", bufs=2)
                nc.sync.dma_start(dens[:], scr[:])
                rec = phB.tile([H_CORE, QT], f32, tag="rc", bufs=2)
                nc.vector.reciprocal(rec[:], dens[:])
                scr2 = dscr.tile([H_CORE, QT], f32, name=f"scr2_{qh}")
                nc.sync.dma_start(scr2[:], rec[:])
                for h, u65 in us:
                    bc = phB.tile([D, QT], f32, tag="B")
                    nc.sync.dma_start(
                        bc[:], scr2[h:h + 1, :].to_broadcast((D, QT))
                    )
                    pb = 64 * (h % 2)
                    nc.vector.tensor_tensor(
                        attn_t[pb:pb + D, h // 2, qh * QT:(qh + 1) * QT],
                        u65[0:D, :],
                        bc[:],
                        ALU.mult,
                    )

            # ---------------- output projection (tail) ----------------
            # Emitted after all attention so its PSUM-slot reservations never
            # block attention allocations (slot grants are FIFO per tag).
            # A short warmup burst re-warms the PE after the norm-chain gap.
            wu2 = psB.tile([P, QT], f32, tag="S", name="warmup2")
            for i in range(12):
                nc.tensor.matmul(
                    wu2[:, 0:MMF], wo_t[:, 0, 0:P], wo_t[:, 0, 0:MMF],
                    start=True, stop=True,
                )
            for qh in range(NQH):
                for ot in range(C_CHUNKS):
                    ps = psB.tile([P, QT], f32, tag="S")
                    for ct in range(QTILES):
                        for nf in range(NF):
                            nc.tensor.matmul(
                                ps[:, nf * MMF:(nf + 1) * MMF],
                                wo_t[:, ct, ot * P:(ot + 1) * P],
                                attn_t[:, ct, qh * QT + nf * MMF:
                                       qh * QT + (nf + 1) * MMF],
                                start=(ct == 0),
                                stop=(ct == QTILES - 1),
                            )
                    o_sb = outp.tile([P, QT], f32, tag="O")
                    nc.vector.tensor_scalar_add(
                        o_sb[:], ps[:], bo_t[:, ot:ot + 1]
                    )
                    nc.sync.dma_start(
                        out_r[:, ot, qh * QT:(qh + 1) * QT], o_sb[:]
                    )

    nc.compile()
    return nc


def _get_program(with_mask: bool):
    key = ("prog", with_mask)
    if key not in _CACHE:
        _CACHE[key] = _build(with_mask)
    return _CACHE[key]


def _prep_inputs(hidden_state, mask, Wq, bq, Wk, bk, Wv, bv, Wo, bo):
    """Build the 8 per-core input dicts (host-side shard + weight prep)."""
    f = np.float32
    scale = np.float32(D ** -0.5)
    with_mask = bool(np.any(mask))

    in_maps = []
    for b in range(B):
        x_b = _round_f32r(hidden_state[b, :, 0, :])
        if with_mask:
            em_b = np.exp(mask[b, :, 0, :].astype(f))
        for g in range(2):
            rows = slice(OC * g, OC * (g + 1))
            wqT = _round_f32r((np.asarray(Wq[rows, :], dtype=f) * scale).T)
            bqs = np.ascontiguousarray(np.asarray(bq[rows], dtype=f) * scale)
            wkT = _round_f32r(np.asarray(Wk[rows, :], dtype=f).T)
            bks = np.ascontiguousarray(bk[rows], dtype=f)
            # augmented v weights: col 65h+j = Wv row, col 65h+64 = 0 (bias 1)
            wvT = np.zeros((HIDDEN, WAUG), dtype=f)
            bvb = np.zeros((WAUG,), dtype=f)
            for h in range(H_CORE):
                wvT[:, 65 * h:65 * h + 64] = np.asarray(
                    Wv[OC * g + D * h:OC * g + D * h + D, :], dtype=f).T
                bvb[65 * h:65 * h + 64] = bv[OC * g + D * h:OC * g + D * h + D]
                bvb[65 * h + 64] = 1.0
            woT = _round_f32r(np.asarray(Wo[:, rows], dtype=f).T)
            m = {
                "x": x_b,
                "wqT": wqT,
                "bq": bqs,
                "wkT": wkT,
                "bk": bks,
                "wvT": _round_f32r(wvT),
                "bvb": np.broadcast_to(bvb, (P, WAUG)).copy(),
                "woT": woT,
                "bo": (np.asarray(bo, dtype=f) * np.float32(0.5)),
            }
            if with_mask:
                m["expmask"] = em_b
            in_maps.append(m)
    return in_maps, with_mask


def run(inputs: dict, trace: bool = False):
    """Run on 8 NeuronCores; returns (full_output, exec_time_ns_or_None)."""
    from concourse import bass_utils

    in_maps, with_mask = _prep_inputs(**inputs)
    nc = _get_program(with_mask)
    res = bass_utils.run_bass_kernel_spmd(
        nc, in_maps, core_ids=list(range(8)), trace=trace
    )
    out = np.empty((B, HIDDEN, 1, S), dtype=np.float32)
    for b in range(B):
        out[b, :, 0, :] = res.results[2 * b]["out"] + res.results[2 * b + 1]["out"]
    return out, res.exec_time_ns


def kernel(**inputs) -> np.ndarray:
    out, _ = run(inputs, trace=False)
    return out


# revision 4
# speedup vs baseline: 1.2636x; 1.2636x over previous
"""Trainium2 Bass kernel for nn_Attention (B=4, S=2048, HIDDEN=768, 12 heads).

Sharding: 8 cores = 4 batches x 2 head-groups (6 heads each). Projection
weights are sliced per head-group and pre-transposed on the host. Each core
computes a partial output (its head-group's contribution through Wo, with
bo/2 bias); the host sums the two partials per batch.

q/k projections run in fp8e4 DoubleRow mode (two 128-channel chunks per
pass, 2 MACs/cell/cycle). fp8e4 normals bottom out at 2^-6, so the host
scales Wq/Wk (std 0.02) by 32 before quantizing; the 1/(32*32) and the
attention 1/sqrt(64) are folded into the exp activation's free scale
multiplier (2^-13 exactly). The v projection and the attention inner
products stay bf16: v noise feeds the softmax averaging amplification
directly, and all-bf16 attention keeps PE duty high enough that the HAM
clock gate stays at full rate.

Per-core device program:
  warmup: dense matmul burst at kernel start un-throttles the PE HAM clock
  q,k  : [384, S] fp8 DoubleRow, 8 concurrent PSUM groups track x DMA
  vT   : [S, 390] bf16, computed directly transposed; augmented all-zero
         weight column with bias 1.0 appends a ones-column per head
  scores S_c = k_chunk^T q -> PSUM [128, QT]   (k on partitions, q on free)
  E_c = exp(2^-13 * S_c) on ScalarE, PSUM -> SBUF bf16
  A  += [vT_c | 1]^T @ E_c -> PSUM [65, QT]    (row 64 = softmax denominator)
  u65 = copy(A) -> SBUF (single 65-row evacuation frees PSUM fast)
  den rows DMA-gathered to DRAM; one batched reciprocal [6, QT]; DMA
  broadcast back; attn = u65[0:64] * bcast(1/den)
  out_partial = WoT_g^T @ attn + bo/2
"""

import numpy as np
import ml_dtypes

HIDDEN = 768
NUM_HEADS = 12
D = 64
B = 4
S = 2048
P = 128

H_CORE = 6          # heads per core
OC = H_CORE * D     # 384 output channels per core for q/k
WAUG = H_CORE * (D + 1)  # 390: v columns with interleaved ones-columns
C_CHUNKS = HIDDEN // P   # 6
TC = HIDDEN // 256  # 3 DoubleRow contraction chunks (256 channels each)
QT = 1024           # q-tile (free dim) for the attention inner loop
MMF = 512           # max fp32 moving free dim for a PSUM-bank matmul
WSCALE = 32.0       # fp8 pre-scale for Wq/Wk (keeps weights out of subnormals)
EXP_SCALE = float(2.0 ** -13)  # 1/(32*32) * 1/sqrt(64)

_CACHE = {}


def _build(with_mask: bool):
    import concourse.bass as bass
    import concourse.tile as tile
    from concourse import bacc, mybir
    from contextlib import ExitStack

    f32 = mybir.dt.float32
    bf16 = mybir.dt.bfloat16
    f8 = mybir.dt.float8e4
    AF = mybir.ActivationFunctionType
    ALU = mybir.AluOpType
    DR = mybir.MatmulPerfMode.DoubleRow

    nc = bacc.Bacc(
        "TRN2",
        target_bir_lowering=False,
        debug=False,
        enable_asserts=True,
        num_devices=8,
    )

    # x8: fp8 x in DoubleRow layout [p, t, j, s] = x[256t + 128j + p, s]
    x8_d = nc.dram_tensor("x8", (P, TC, 2, S), f8, kind="ExternalInput").ap()
    # xb: bf16 x chunks [p, c, s] = x[128c + p, s] (v-projection stationary)
    xb_d = nc.dram_tensor("xb", (P, C_CHUNKS, S), bf16, kind="ExternalInput").ap()
    wq_d = nc.dram_tensor("wq8", (P, TC, 2, OC), f8, kind="ExternalInput").ap()
    bq_d = nc.dram_tensor("bq", (OC,), f32, kind="ExternalInput").ap()
    wk_d = nc.dram_tensor("wk8", (P, TC, 2, OC), f8, kind="ExternalInput").ap()
    bk_d = nc.dram_tensor("bk", (OC,), f32, kind="ExternalInput").ap()
    wv_d = nc.dram_tensor("wvT", (HIDDEN, WAUG), bf16, kind="ExternalInput").ap()
    bvb_d = nc.dram_tensor("bvb", (P, WAUG), f32, kind="ExternalInput").ap()
    wo_d = nc.dram_tensor("woT", (OC, HIDDEN), bf16, kind="ExternalInput").ap()
    bo_d = nc.dram_tensor("bo", (HIDDEN,), f32, kind="ExternalInput").ap()
    if with_mask:
        em_d = nc.dram_tensor("expmask", (S, S), f32, kind="ExternalInput").ap()
    out_d = nc.dram_tensor("out", (HIDDEN, S), f32, kind="ExternalOutput").ap()

    QTILES = OC // P      # 3 q/k sbuf tiles
    STILES = S // P       # 16 s-position chunks
    NQH = S // QT         # 2 q-halves
    NF = QT // MMF        # 2 matmul free-slices per QT
    NSL = S // MMF        # 4 512-slices of S

    wv_r = wv_d.rearrange("(t p) o -> p t o", p=P)
    wo_r = wo_d.rearrange("(t p) o -> p t o", p=P)
    bq_r = bq_d.rearrange("(t p) -> p t", p=P)
    bk_r = bk_d.rearrange("(t p) -> p t", p=P)
    bo_r = bo_d.rearrange("(t p) -> p t", p=P)
    out_r = out_d.rearrange("(t p) s -> p t s", p=P)

    with tile.TileContext(nc) as tc, ExitStack() as ctx:
        consts = ctx.enter_context(tc.tile_pool(name="consts", bufs=1))
        persist = ctx.enter_context(tc.tile_pool(name="persist", bufs=1))

        bvb_t = consts.tile([P, WAUG], f32)
        nc.sync.dma_start(bvb_t[:], bvb_d)
        bq_t = consts.tile([P, QTILES], f32)
        nc.sync.dma_start(bq_t[:], bq_r)
        bk_t = consts.tile([P, QTILES], f32)
        nc.sync.dma_start(bk_t[:], bk_r)
        bo_t = consts.tile([P, C_CHUNKS], f32)
        nc.sync.dma_start(bo_t[:], bo_r)

        q_t = persist.tile([P, QTILES, S], bf16)
        k_t = persist.tile([P, QTILES, S], bf16)
        v_t = persist.tile([P, STILES, WAUG], bf16)
        attn_t = persist.tile([P, QTILES, S], bf16)
        wo_t = persist.tile([P, QTILES, HIDDEN], bf16)

        # ---------------- phase A: projections ----------------
        with (
            tc.tile_pool(name="phA", bufs=1) as phA,
            tc.tile_pool(name="psA", bufs=8, space="PSUM") as psA,
        ):
            # Startup warmup: the PE starts HAM-throttled at 1.2 GHz and only
            # un-throttles after ~3.4us of sustained activity. Burn that time
            # on fp32 matmuls (4 passes each) while the x/w DMAs land.
            wu0 = psA.tile([P, MMF], f32, tag="W", name="warmup0")
            for i in range(5):
                nc.tensor.matmul(
                    wu0[:, 0:390], bvb_t[:, 0:P], bvb_t[:],
                    start=True, stop=True,
                )

            x8_t = phA.tile([P, TC, 2, S], f8)
            xb_t = phA.tile([P, C_CHUNKS, S], bf16)
            wq_t = phA.tile([P, TC, 2, OC], f8)
            wk_t = phA.tile([P, TC, 2, OC], f8)
            wv_t = phA.tile([P, C_CHUNKS, WAUG], bf16)
            nc.sync.dma_start(wq_t[:], wq_d)
            nc.sync.dma_start(wk_t[:], wk_d)
            for t in range(TC):
                nc.sync.dma_start(x8_t[:, t, :, :], x8_d[:, t, :, :])
            for c in range(C_CHUNKS):
                nc.sync.dma_start(xb_t[:, c, :], xb_d[:, c, :])
                nc.sync.dma_start(wv_t[:, c, :], wv_r[:, c, :])
            nc.sync.dma_start(wo_t[:], wo_r)

            # q/k projections, fp8 DoubleRow: 8 concurrent [128, 512] PSUM
            # groups; each group accumulates 3 DoubleRow passes over the
            # 768-channel contraction. Waves ordered so q/k tile 0 finish
            # first.
            groups = [(dst, w_sb, b_sb, ot, sl)
                      for ot in range(QTILES)
                      for dst, w_sb, b_sb in ((q_t, wq_t, bq_t),
                                              (k_t, wk_t, bk_t))
                      for sl in range(NSL)]
            for wave in range(3):
                gslice = groups[8 * wave:8 * wave + 8]
                tiles = [psA.tile([P, MMF], f32, tag="W",
                                  name=f"pw{wave}_{i}")
                         for i in range(len(gslice))]
                for t in range(TC):
                    for ps, (dst, w_sb, b_sb, ot, sl) in zip(tiles, gslice):
                        nc.tensor.matmul(
                            ps[:],
                            w_sb[:, t, :, ot * P:(ot + 1) * P],
                            x8_t[:, t, :, sl * MMF:(sl + 1) * MMF],
                            start=(t == 0),
                            stop=(t == TC - 1),
                            perf_mode=DR,
                        )
                for ps, (dst, w_sb, b_sb, ot, sl) in zip(tiles, gslice):
                    nc.vector.tensor_scalar_add(
                        dst[:, ot, sl * MMF:(sl + 1) * MMF],
                        ps[:],
                        b_sb[:, ot:ot + 1],
                    )

            # vT projection (bf16): out[s_tile(128), 390] = sum_c x^T WvT
            for st in range(STILES):
                ps = psA.tile([P, MMF], f32, tag="W")
                for c in range(C_CHUNKS):
                    nc.tensor.matmul(
                        ps[:, 0:WAUG],
                        xb_t[:, c, st * P:(st + 1) * P],
                        wv_t[:, c, :],
                        start=(c == 0),
                        stop=(c == C_CHUNKS - 1),
                    )
                nc.vector.tensor_tensor(
                    v_t[:, st, :], ps[:, 0:WAUG], bvb_t[:], ALU.add
                )

        # ---------------- phase B: attention ----------------
        with (
            tc.tile_pool(name="phB", bufs=4) as phB,
            tc.tile_pool(name="psB", bufs=2, space="PSUM") as psB,
            tc.tile_pool(name="outp", bufs=2) as outp,
            tc.tile_pool(name="dscr", bufs=2, space="DRAM") as dscr,
        ):
            # re-warm after the proj->attention boundary
            wu = psB.tile([P, QT], f32, tag="S", name="warmup")
            for i in range(14):
                nc.tensor.matmul(
                    wu[:, 0:MMF], q_t[:, 0, 0:P], q_t[:, 0, 0:MMF],
                    start=True, stop=True,
                )
            for qh in range(NQH):
                us = []
                scr = dscr.tile([H_CORE, QT], f32, name=f"scr{qh}")
                for hp in range(H_CORE // 2):
                    heads = (2 * hp, 2 * hp + 1)
                    accs = [
                        psB.tile([D + 1, QT], f32, tag="A", name=f"acc{i}")
                        for i in range(2)
                    ]
                    for c in range(STILES):
                        etiles = []
                        for hi, h in enumerate(heads):
                            pb = 64 * (h % 2)
                            sc = psB.tile([P, QT], f32, tag="S")
                            for nf in range(NF):
                                nc.tensor.matmul(
                                    sc[:, nf * MMF:(nf + 1) * MMF],
                                    k_t[pb:pb + D, h // 2, c * P:(c + 1) * P],
                                    q_t[pb:pb + D, h // 2,
                                        qh * QT + nf * MMF:
                                        qh * QT + (nf + 1) * MMF],
                                    start=True,
                                    stop=True,
                                )
                            e = phB.tile([P, QT], bf16, tag="E")
                            nc.scalar.activation(e[:], sc[:], AF.Exp,
                                                 scale=EXP_SCALE)
                            if with_mask:
                                em = phB.tile([P, QT], f32, tag="M")
                                nc.sync.dma_start(
                                    em[:],
                                    em_d[c * P:(c + 1) * P,
                                         qh * QT:(qh + 1) * QT],
                                )
                                nc.vector.tensor_tensor(
                                    e[:], e[:], em[:], ALU.mult
                                )
                            etiles.append(e)
                        for hi, h in enumerate(heads):
                            for nf in range(NF):
                                nc.tensor.matmul(
                                    accs[hi][:, nf * MMF:(nf + 1) * MMF],
                                    v_t[:, c, 65 * h:65 * h + 65],
                                    etiles[hi][:, nf * MMF:(nf + 1) * MMF],
                                    start=(c == 0),
                                    stop=(c == STILES - 1),
                                )
                    for hi, h in enumerate(heads):
                        # single 65-row evacuation frees the acc PSUM fast;
                        # row 64 (softmax denominator) is DMA-gathered to DRAM
                        u65 = phB.tile([D + 1, QT], f32, tag="U", bufs=7,
                                       name=f"u{h}")
                        nc.vector.tensor_copy(u65[:], accs[hi][:])
                        nc.sync.dma_start(scr[h:h + 1, :], u65[D:D + 1, :])
                        us.append((h, u65))
                    # hp-boundary warmup: the acc-tag FIFO stalls the PE here
                    # long enough for HAM to re-throttle; keep it busy.
                    wub = psB.tile([P, QT], f32, tag="S", name=f"wub{qh}_{hp}")
                    for i in range(8):
                        nc.tensor.matmul(
                            wub[:, 0:MMF], q_t[:, 0, 0:P], q_t[:, 0, 0:MMF],
                            start=True, stop=True,
                        )
                # batched reciprocal of all 6 denominators of this qh
                dens = phB.tile([H_CORE, QT], f32, tag="dn", bufs=2)
                nc.sync.dma_start(dens[:], scr[:])
                rec = phB.tile([H_CORE, QT], f32, tag="rc", bufs=2)
                nc.vector.reciprocal(rec[:], dens[:])
                scr2 = dscr.tile([H_CORE, QT], f32, name=f"scr2_{qh}")
                nc.sync.dma_start(scr2[:], rec[:])
                for h, u65 in us:
                    bc = phB.tile([D, QT], f32, tag="B")
                    nc.sync.dma_start(
                        bc[:], scr2[h:h + 1, :].to_broadcast((D, QT))
                    )
                    pb = 64 * (h % 2)
                    nc.vector.tensor_tensor(
                        attn_t[pb:pb + D, h // 2, qh * QT:(qh + 1) * QT],
                        u65[0:D, :],
                        bc[:],
                        ALU.mult,
                    )

            # ---------------- output projection (tail) ----------------
            # Emitted after all attention so its PSUM-slot reservations never
            # block attention allocations (slot grants are FIFO per tag).
            # A short warmup burst re-warms the PE after the norm-chain gap.
            wu2 = psB.tile([P, QT], f32, tag="S", name="warmup2")
            for i in range(12):
                nc.tensor.matmul(
                    wu2[:, 0:MMF], q_t[:, 0, 0:P], q_t[:, 0, 0:MMF],
                    start=True, stop=True,
                )
            for qh in range(NQH):
                for ot in range(C_CHUNKS):
                    ps = psB.tile([P, QT], f32, tag="S")
                    for ct in range(QTILES):
                        for nf in range(NF):
                            nc.tensor.matmul(
                                ps[:, nf * MMF:(nf + 1) * MMF],
                                wo_t[:, ct, ot * P:(ot + 1) * P],
                                attn_t[:, ct, qh * QT + nf * MMF:
                                       qh * QT + (nf + 1) * MMF],
                                start=(ct == 0),
                                stop=(ct == QTILES - 1),
                            )
                    o_sb = outp.tile([P, QT], f32, tag="O")
                    nc.vector.tensor_scalar_add(
                        o_sb[:], ps[:], bo_t[:, ot:ot + 1]
                    )
                    nc.sync.dma_start(
                        out_r[:, ot, qh * QT:(qh + 1) * QT], o_sb[:]
                    )

    nc.compile()
    return nc


def _get_program(with_mask: bool):
    key = ("prog", with_mask)
    if key not in _CACHE:
        _CACHE[key] = _build(with_mask)
    return _CACHE[key]


def _prep_inputs(hidden_state, mask, Wq, bq, Wk, bk, Wv, bv, Wo, bo):
    """Build the 8 per-core input dicts (host-side shard + weight prep)."""
    f = np.float32
    f8 = ml_dtypes.float8_e4m3
    bf = ml_dtypes.bfloat16
    with_mask = bool(np.any(mask))
    ws = np.float32(WSCALE)

    def dr_layout(wT):
        # [HIDDEN, O] -> [P, TC, 2, O] with channel (256t + 128j + p)
        return np.ascontiguousarray(
            wT.reshape(TC, 2, P, -1).transpose(2, 0, 1, 3))

    in_maps = []
    for b in range(B):
        x_b = np.asarray(hidden_state[b, :, 0, :], dtype=f)
        x8 = dr_layout(x_b).astype(f8)
        xb = np.ascontiguousarray(
            x_b.reshape(C_CHUNKS, P, S).transpose(1, 0, 2)).astype(bf)
        if with_mask:
            em_b = np.exp(mask[b, :, 0, :].astype(f))
        for g in range(2):
            rows = slice(OC * g, OC * (g + 1))
            wq8 = dr_layout(np.asarray(Wq[rows, :], dtype=f).T * ws).astype(f8)
            bqs = np.ascontiguousarray(np.asarray(bq[rows], dtype=f) * ws)
            wk8 = dr_layout(np.asarray(Wk[rows, :], dtype=f).T * ws).astype(f8)
            bks = np.ascontiguousarray(np.asarray(bk[rows], dtype=f) * ws)
            # augmented v weights: col 65h+j = Wv row, col 65h+64 = 0 (bias 1)
            wvT = np.zeros((HIDDEN, WAUG), dtype=f)
            bvb = np.zeros((WAUG,), dtype=f)
            for h in range(H_CORE):
                wvT[:, 65 * h:65 * h + 64] = np.asarray(
                    Wv[OC * g + D * h:OC * g + D * h + D, :], dtype=f).T
                bvb[65 * h:65 * h + 64] = bv[OC * g + D * h:OC * g + D * h + D]
                bvb[65 * h + 64] = 1.0
            woT = (np.asarray(Wo[:, rows], dtype=f).T).astype(bf)
            m = {
                "x8": x8,
                "xb": xb,
                "wq8": wq8,
                "bq": bqs,
                "wk8": wk8,
                "bk": bks,
                "wvT": wvT.astype(bf),
                "bvb": np.broadcast_to(bvb, (P, WAUG)).copy(),
                "woT": woT,
                "bo": (np.asarray(bo, dtype=f) * np.float32(0.5)),
            }
            if with_mask:
                m["expmask"] = em_b
            in_maps.append(m)
    return in_maps, with_mask


def run(inputs: dict, trace: bool = False):
    """Run on 8 NeuronCores; returns (full_output, exec_time_ns_or_None)."""
    from concourse import bass_utils

    in_maps, with_mask = _prep_inputs(**inputs)
    nc = _get_program(with_mask)
    res = bass_utils.run_bass_kernel_spmd(
        nc, in_maps, core_ids=list(range(8)), trace=trace
    )
    out = np.empty((B, HIDDEN, 1, S), dtype=np.float32)
    for b in range(B):
        out[b, :, 0, :] = res.results[2 * b]["out"] + res.results[2 * b + 1]["out"]
    return out, res.exec_time_ns


def kernel(**inputs) -> np.ndarray:
    out, _ = run(inputs, trace=False)
    return out
